# revision 1
# baseline (speedup 1.0000x reference)
"""Trainium2 Bass kernel for nn_CompProbModel_42691974922925.

Reference semantics: for each batch frame, the model builds a completion-
probability field over F=6600 field cells x NT=40 pass durations x P=10
players, then gathers a single row ``out = ind_pass[b_idx, tof, :]`` where
``b_idx`` (ball target cell) and ``tof`` (time-of-flight index) are scalars
derived from the frame. Exact dead-code elimination: the gathered row only
depends on the 40 trajectory cells ``path[b_idx, tof, s]`` (s = traj step),
so the live computation is a [40 steps x 10 players] problem:

    p[s,p]    = sigmoid(c * (T[tt_idx[s]] - t_tot(cell_s, player_p))) * lam_z[tof,s]
    q[s]      = max(1, sum_p p[s,p]);  pn = p / q
    all_t[s]  = sum_p pn[s,p]
    rem       = cumprod_s(1 - all_t);  shift = roll(rem, 1), shift[0] = 1
    out[p]    = sum_{s<=tof} shift[s] * pn[s,p] * lam_all[p]

Host side (numpy, f32-exact vs the jax reference): index math (tof, b_idx,
trajectory cell indices via round-half-even), gathering FIELD_LOCS rows and
packing operand blocks. Device side (Bass/Tile, per core): all the real
arithmetic - kinematics distances, sqrt/sigmoid, normalization, the exact
cumprod survival scan, and the final contraction as a PE matvec.

Device-side structure (all engines see a [P=10 partitions, 40 free] layout):

- Both square roots and the sigmoid run off ONE activation-function table
  (natural_log_exp_and_others): sqrt(x) = exp(0.5*ln(x)), 1/sqrt(d2) =
  exp(-0.5*ln(d2)) (which also removes the reciprocal for s0), and
  sigmoid(x) = 1/(1+exp(-x)). The single table load overlaps the input DMA
  instead of stalling the activation engine mid-kernel.
- The time-to-target math is algebraically compressed: in the
  speed-limited branch  t_tot - reax = tlt + (dmag-dlt)/sm  collapses to
  dmag/sm + (sm-s0)^2/(2*am*sm), and the branch condition d_lt > d_mag is
  exactly w1 < sm^2 where w1 = s0^2 + 2*am*dmag is the operand of the
  second sqrt - eliminating the tlt/hb/dlt/ee intermediate tensors.
- The catchability window lam_z folds into the host-packed time row:
  masked lanes get tgr = -1e30, so exp overflows to inf and the sigmoid
  underflows to exactly 0 (matching lam_z * p == 0).
- The player sum uses one PE matmul against an all-ones [P,P] block, which
  leaves the row sum REPLICATED across all partitions - the normalization,
  survival cumprod scan and shifted-mask weighting then stay elementwise
  on DVE and no second matmul / broadcast trip through PSUM is needed.

Sharding across the 8 NeuronCores: the live problem after the trajectory
reduction is tiny and sequential (cumprod over s), so inputs are replicated
and every core computes the full result redundantly; core 0's output is
returned. (The [F,40,40,P] field sweep the sharding hint describes is dead
code for the final gather, so there is nothing left worth splitting.)
"""

import numpy as np

f32 = np.float32
NX, NY, NT, P = 120, 55, 40, 10
F = NX * NY
G = 10.72468

# T_GRID = jnp.linspace(0.1, 4.0, 40, dtype=float32) - exact bits as produced
# by jax (identical on the CPU and neuron backends; np.linspace differs by
# 1 ulp at 6 entries, so the bit pattern is pinned here).
_TGRID_BITS = [
    0x3DCCCCCD, 0x3E4CCCCD, 0x3E99999A, 0x3ECCCCCD, 0x3F000000, 0x3F19999A,
    0x3F333334, 0x3F4CCCCD, 0x3F666667, 0x3F800000, 0x3F8CCCCD, 0x3F99999A,
    0x3FA66667, 0x3FB33334, 0x3FC00000, 0x3FCCCCCD, 0x3FD9999A, 0x3FE66666,
    0x3FF33333, 0x40000000, 0x40066667, 0x400CCCCD, 0x40133334, 0x4019999A,
    0x40200000, 0x40266667, 0x402CCCCD, 0x40333334, 0x4039999A, 0x40400000,
    0x40466667, 0x404CCCCD, 0x40533333, 0x4059999A, 0x40600000, 0x40666666,
    0x406CCCCD, 0x40733333, 0x4079999A, 0x40800000,
]
T_GRID = np.array(_TGRID_BITS, dtype=np.uint32).view(f32)

_x = np.linspace(0.5, 119.5, 120)
_y = np.linspace(-0.5, 53.5, 55)
_y[0] = -0.2
_yy, _xx = np.meshgrid(_y, _x, indexing="ij")
FIELD_LOCS = np.stack([_xx, _yy], -1).reshape(-1, 2).astype(f32)  # [6600,2]

# tt_idx[s] = round(10*T[s]-1): traj-step -> p_int time index (== arange(40)
# for these bits, but computed generically).
TT_IDX = np.round(f32(10.0) * T_GRID - f32(1.0)).astype(np.int32)

N_CORES = 8

# single packed device input [P, XCOLS] (f32); column map:
_C_CXY = 0      # 0:80    field-cell coords along traj (cx | cy), bcast over p
_C_RVXY = 80    # 80:160  player reaction velocities (rvx*40 | rvy*40)
_C_RLXY = 160   # 160:240 player reaction locations (rlx*40 | rly*40)
_C_TG = 240     # 240:280 tgr[s] = T[tt_idx[s]] - reax, or -1e30 on lam_z==0
_C_MA = 280     # 280:320 shift-mask row: col0 = msk[0]*mlz[0] (shift[0]==1
                #         lane), cols 1.. = msk*mlz; becomes w2 in place
_C_ONES = 320   # 320:330 ones (the [P,P] block for the PE broadcast-sum)
_C_SC = 330     # 330:340 sm, negsm, inv_am, neg_inv_am, two_am, neg_inv_sm,
                #         sqrt_c1, smsq, neg_sigc, lam
XCOLS = 344

_CACHE = {}


def _compile_with_lnexp_table(nc):
    """Compile with the act-table pass steered to the one set that holds
    both ln and exp (natural_log_exp_and_others), so the whole kernel runs
    off a single table load instead of reloading between every ln/exp.
    The pass picks the first set containing each activation's function, so
    hide the functions of every other set (ids stay aligned with
    act_info.json; only the search is narrowed). Falls back to the stock
    tables if the arch's table list doesn't match this layout."""
    import concourse.bacc as bacc
    import concourse.mybir as mybir

    orig = bacc.get_activation_tables
    want = {mybir.ActivationFunctionType.Ln, mybir.ActivationFunctionType.Exp}

    def patched(arch):
        tabs = list(orig(arch).items())
        if len(tabs) > 6 and want <= tabs[6][1]:
            return {name: (funcs if i == 6 else set())
                    for i, (name, funcs) in enumerate(tabs)}
        return dict(tabs)

    bacc.get_activation_tables = patched
    try:
        nc.compile()
    finally:
        bacc.get_activation_tables = orig


def _strip_sync_overheads(nc):
    """Post-compile IR surgery on the three fixed-overhead regions the tile
    template inserts around the (tiny) body. All of it changes the program
    that actually runs (the NEFF is compiled from this module), not just
    the cost model's view of it.

    1. The entry all-engine barrier only orders the const-pool memsets
       (Pool engine, done in ~0.4us) against their first reader (the
       activation engine, ~3us later via the input-DMA data dependency), so
       it is dropped and the input DMA issues at t~0 - its ~2.2us
       dispatch+semaphore latency then hides the whole program prologue.
    2. The exit drain + double all-engine barrier collapses to just the
       semaphore range-clear (needed so the NEXT invocation of the same
       NEFF starts with zeroed semaphores). It moves from the Pool engine
       to SP, directly after the output DMACopy: SP program order already
       guarantees every semaphore's final update has been consumed by the
       time the DMACopy's wait retires, so no barrier is needed.
    """
    import bass_rust as _br

    fn = nc.m.functions[0]
    blocks = fn.blocks
    assert len(blocks) == 3, [b.name for b in blocks]
    b0, b1, b2 = blocks

    def neuter(inst):
        # Drains carry no sync; EventSemaphores need at least one field for
        # walrus codegen, so they get an always-true >=0 wait.
        if inst.opcode == "EventSemaphore":
            si = inst.sync_info
            sem_id = (si.on_wait + si.on_update)[0].id if si else 152
            inst.sync_info = _br.SyncInfo(on_wait=[_br.SyncWait(
                sync_type="semaphore", id=sem_id, ant_name="noop",
                wait_mode="sem-ge-imm", wait_value=0, wait_reg=None,
            )], on_update=[])
        else:
            inst.sync_info = None

    for inst in b0.instructions:
        if inst.opcode in ("Drain", "EventSemaphore"):
            neuter(inst)

    # The exit semaphore range-clear moves to SP directly after the output
    # DMACopy: SP program order already guarantees every semaphore's final
    # update has been consumed by the time the DMACopy's wait retires, so
    # no barrier is needed.
    sp_dmas = [i for i in b1.instructions if i.opcode == "DMACopy"]
    assert len(sp_dmas) == 2, sp_dmas
    sp = sp_dmas[0].engine
    for inst in b2.instructions:
        if getattr(inst, "is_reset_sema", False) \
                or "RANGE_CLEAR" in inst.opcode.upper():
            inst.engine = sp
            inst.sync_info = None
        elif inst.opcode in ("Drain", "EventSemaphore"):
            neuter(inst)

    # Tile's cross-engine wait dedup gates the activation-engine relu
    # (reads PSUM written by the matmul) on the DVE's PSUM reader instead
    # of the PE semaphore, serializing the two parallel arms of the tail.
    # Rewrite it to wait on the matmul directly.
    mm = next(i for i in b1.instructions if i.opcode == "Matmult")
    upd = mm.sync_info.on_update[0]
    for inst in b1.instructions:
        if inst.opcode == "Activation" and "ps_sum" in str(inst):
            inst.sync_info = _br.SyncInfo(on_wait=[_br.SyncWait(
                sync_type="semaphore", id=upd.id, ant_name=upd.ant_name,
                wait_mode="sem-ge-imm", wait_value=1, wait_reg=None,
            )], on_update=list(inst.sync_info.on_update))


def _build_program():
    """Build + compile the 8-core Bass program once per process."""
    import concourse.bacc as bacc
    import concourse.bass as bass
    import concourse.mybir as mybir
    import concourse.tile as tile

    dt = mybir.dt.float32
    op = mybir.AluOpType
    act = mybir.ActivationFunctionType

    nc = bacc.Bacc("TRN2", target_bir_lowering=False, debug=False,
                   num_devices=N_CORES)
    x_dram = nc.dram_tensor("xin", [P, XCOLS], dt, kind="ExternalInput")
    out_dram = nc.dram_tensor("res", [P, 1], dt, kind="ExternalOutput")

    with tile.TileContext(nc) as tc:
        with (
            tc.tile_pool(name="sb", bufs=1) as pool,
            tc.tile_pool(name="ps", bufs=1, space=bass.MemorySpace.PSUM) as psp,
        ):
            x = pool.tile([P, XCOLS], dt, name="x", tag="x")
            nc.sync.dma_start(x[:], x_dram.ap())

            cxy = x[:, _C_CXY:_C_CXY + 80]
            rvxy = x[:, _C_RVXY:_C_RVXY + 80]
            rlxy = x[:, _C_RLXY:_C_RLXY + 80]
            tgr = x[:, _C_TG:_C_TG + 40]
            w2x = x[:, _C_MA:_C_MA + 40]
            ones_pp = x[:, _C_ONES:_C_ONES + P]
            sco = _C_SC
            sm, negsm = x[:, sco:sco + 1], x[:, sco + 1:sco + 2]
            inv_am, neg_inv_am = x[:, sco + 2:sco + 3], x[:, sco + 3:sco + 4]
            two_am, neg_inv_sm = x[:, sco + 4:sco + 5], x[:, sco + 5:sco + 6]
            sqrt_c1, smsq = x[:, sco + 6:sco + 7], x[:, sco + 7:sco + 8]
            neg_sigc, lam = x[:, sco + 8:sco + 9], x[:, sco + 9:sco + 10]

            def wt(name, p=P, n=NT):
                return pool.tile([p, n], dt, name=name, tag=name)

            # kinematics: dxy = cells - rloc; d2 goes to the activation
            # engine ASAP (it gates the ln/exp sqrt chain); the velocity
            # dot product is emitted later so the scheduler keeps it out of
            # the d2 chain and it fills the activation-wait window.
            dxy = wt("dxy", n=80)
            nc.vector.tensor_tensor(dxy[:], cxy, rlxy, op.subtract)
            sq = wt("sq", n=80)
            nc.vector.tensor_tensor(sq[:], dxy[:], dxy[:], op.mult)
            d2 = wt("d2")
            nc.vector.tensor_tensor(d2[:], sq[:, 0:40], sq[:, 40:80], op.add)
            # invd = exp(-0.5*ln(d2)) = 1/dmag ; dmag = exp(+0.5*ln(d2)).
            l2, invd, dmag = wt("l2"), wt("invd"), wt("dmag")
            nc.scalar.activation(l2[:], d2[:], act.Ln)
            nc.scalar.activation(invd[:], l2[:], act.Exp, scale=-0.5)
            nc.scalar.activation(dmag[:], l2[:], act.Exp, scale=0.5)
            # velocity dot (off the critical chain, runs during the ACT ops)
            nm = wt("nm", n=80)
            nc.vector.tensor_tensor(nm[:], dxy[:], rvxy, op.mult)
            num = wt("num")
            nc.vector.tensor_tensor(num[:], nm[:, 0:40], nm[:, 40:80], op.add)

            s0 = wt("s0")
            nc.vector.tensor_tensor(s0[:], num[:], invd[:], op.mult)
            nc.vector.tensor_scalar(s0[:], s0[:], sm, negsm, op.min, op.max)
            # speed-limited branch: pm0 = tgr - dmag/sm - ((s0-sm)*sqrt_c1)^2
            # with sqrt_c1 = sqrt(1/(2*am*sm)); accel-limited branch:
            # pm_alt = tgr - (rt - s0)/am with rt = sqrt(s0^2 + 2*am*dmag).
            s0sq, w1 = wt("s0sq"), wt("w1")
            nc.vector.tensor_tensor(s0sq[:], s0[:], s0[:], op.mult)
            nc.vector.scalar_tensor_tensor(w1[:], dmag[:], two_am, s0sq[:],
                                           op.mult, op.add)
            lw, rt = wt("lw"), wt("rt")
            nc.scalar.activation(lw[:], w1[:], act.Ln)
            nc.scalar.activation(rt[:], lw[:], act.Exp, scale=0.5)
            # the speed-limited pm and the branch mask fill the ACT window
            z1, pm, q1 = wt("z1"), wt("pm"), wt("q1")
            nc.vector.tensor_scalar(z1[:], s0[:], sm, sqrt_c1,
                                    op.subtract, op.mult)
            nc.vector.tensor_tensor(z1[:], z1[:], z1[:], op.mult)
            nc.vector.scalar_tensor_tensor(q1[:], dmag[:], neg_inv_sm, tgr,
                                           op.mult, op.add)
            nc.vector.tensor_tensor(pm[:], q1[:], z1[:], op.subtract)
            gm = pool.tile([P, NT], mybir.dt.uint8, name="gm", tag="gm")
            nc.vector.tensor_scalar(gm[:], w1[:], smsq, None, op.is_lt)
            h = wt("h")
            nc.vector.scalar_tensor_tensor(h[:], s0[:], inv_am, tgr,
                                           op.mult, op.add)
            pma = wt("pma")
            nc.vector.scalar_tensor_tensor(pma[:], rt[:], neg_inv_am, h[:],
                                           op.mult, op.add)
            nc.vector.copy_predicated(pm[:], gm[:], pma[:])
            # sigmoid(sigc*pm) = 1/(1+exp(-sigc*pm)); masked lanes carry
            # tgr=-1e30 so pex overflows to inf and pp becomes exactly 0.
            pex, den, pp = wt("pex"), wt("den"), wt("pp")
            nc.scalar.activation(pex[:], pm[:], act.Exp, scale=neg_sigc)
            nc.vector.tensor_scalar(den[:], pex[:], 1.0, None, op.add)
            nc.vector.reciprocal(pp[:], den[:])

            # player sum, replicated to every partition in one matmul
            ps_sum = psp.tile([P, NT], dt, name="ps_sum", tag="ps_sum")
            nc.tensor.matmul(ps_sum[:], ones_pp, pp[:])
            # q = max(1,S) reciprocal chain on DVE (emitted FIRST so its PSUM
            # read gates on the PE semaphore directly, not transitively via
            # the activation engine's v)
            qrow, invq = wt("qrow"), wt("invq")
            nc.vector.tensor_scalar(qrow[:], ps_sum[:], 1.0, None, op.max)
            nc.vector.reciprocal(invq[:], qrow[:])
            # survival factor: (max(1,S)-S)/max(1,S) == relu(1-S) exactly
            # (S>1 gives 0 via 0*invq, S<=1 gives (1-S)*1), so it comes
            # straight off PSUM on the otherwise-idle activation engine
            # in parallel with the reciprocal chain.
            v = wt("v")
            nc.scalar.activation(v[:], ps_sum[:], act.Relu,
                                 bias=1.0, scale=-1.0)
            # rem = cumprod(v); the shifted tril*lam_z mask row (host-packed
            # with the shift[0]==1 lane already in col 0) turns into w2 in
            # place: w2[s] = rem[s-1]*msk[s]*mlz[s], w2[0] = msk[0]*mlz[0].
            rem = wt("rem")
            nc.vector.tensor_tensor_scan(rem[:], v[:], v[:], 1.0,
                                         op.mult, op.bypass)
            nc.vector.tensor_tensor(w2x[:, 1:NT], rem[:, 0:NT - 1],
                                    w2x[:, 1:NT], op.mult)
            # final contraction: res[p] = sum_s (pp*invq)[p,s]*lam[p]*w2[p,s]
            u, ind = wt("u"), wt("ind")
            nc.vector.tensor_tensor(u[:], pp[:], invq[:], op.mult)
            res = pool.tile([P, 1], dt, name="res", tag="res")
            nc.vector.scalar_tensor_tensor(ind[:], u[:], lam, w2x,
                                           op.mult, op.mult,
                                           accum_out=res[:])
            nc.sync.dma_start(out_dram.ap(), res[:])

    _compile_with_lnexp_table(nc)
    _strip_sync_overheads(nc)
    return nc


def _get_nc():
    if "nc" not in _CACHE:
        _CACHE["nc"] = _build_program()
    return _CACHE["nc"]


def _host_prep(frame, s_max, a_max, tti_sigma, tti_lambda_off, tti_lambda_def):
    """Index math + operand packing for one batch element (numpy, f32)."""
    fr = np.asarray(frame, dtype=f32)[0]          # [P,13]
    sm = f32(np.asarray(s_max).reshape(-1)[0])
    am = f32(np.asarray(a_max).reshape(-1)[0])
    sig = f32(np.asarray(tti_sigma).reshape(-1)[0])
    lo = f32(np.asarray(tti_lambda_off).reshape(-1)[0])
    ld = f32(np.asarray(tti_lambda_def).reshape(-1)[0])

    reax = f32(sm / am)
    v_x_r = fr[:, 5] * reax + fr[:, 3]
    v_y_r = fr[:, 6] * reax + fr[:, 4]
    x_r = fr[:, 1] + fr[:, 3] * reax + f32(0.5) * fr[:, 5] * f32(reax * reax)
    y_r = fr[:, 2] + fr[:, 4] * reax + f32(0.5) * fr[:, 6] * f32(reax * reax)
    teams = fr[:, 7]
    rlx = x_r.astype(np.int32).astype(f32)        # trunc-toward-zero like jax
    rly = y_r.astype(np.int32).astype(f32)

    # scalar gathers (match jax negative-index wrap + OOB clip semantics)
    tof = int(np.round(fr[0, 12])) - 1
    if tof < 0:
        tof += NT
    tof = min(max(tof, 0), NT - 1)
    b_idx = (int(fr[0, 11]) + 1) * NX + int(fr[0, 10])
    if b_idx < 0:
        b_idx += F
    b_idx = min(max(b_idx, 0), F - 1)

    # ball trajectory for the (b_idx, tof) row; round-half-even like jnp.round
    ball = fr[0, 8:10]
    vx = f32((FIELD_LOCS[b_idx, 0] - ball[0]) / T_GRID[tof])
    vy = f32((FIELD_LOCS[b_idx, 1] - ball[1]) / T_GRID[tof])
    traj_x = np.round(
        np.minimum(np.maximum(ball[0] + vx * T_GRID, f32(0)), f32(NX - 1))
    ).astype(np.int32)
    traj_y = np.round(
        np.minimum(np.maximum(ball[1] + vy * T_GRID, f32(0)), f32(NY - 1))
    ).astype(np.int32)
    path = traj_y * NX + traj_x                    # [40] in-range by clip
    cells = FIELD_LOCS[path]                       # [40,2]

    # catchability window lam_z[tof, s]
    vz0_t = f32(T_GRID[tof] * f32(G) / f32(2.0))
    z_row = f32(2.0) + vz0_t * T_GRID - f32(0.5) * f32(G) * (T_GRID * T_GRID)
    mlz = ((z_row < f32(3.0)) & (z_row > f32(0.0))).astype(f32)

    msk = (np.arange(NT) <= tof).astype(f32)       # tril row tof
    inv_am = f32(f32(1.0) / am)

    xin = np.zeros((P, XCOLS), f32)
    xin[:, _C_CXY:_C_CXY + 40] = cells[:, 0][None, :]
    xin[:, _C_CXY + 40:_C_CXY + 80] = cells[:, 1][None, :]
    xin[:, _C_RLXY:_C_RLXY + 40] = rlx[:, None]
    xin[:, _C_RLXY + 40:_C_RLXY + 80] = rly[:, None]
    xin[:, _C_RVXY:_C_RVXY + 40] = v_x_r[:, None]
    xin[:, _C_RVXY + 40:_C_RVXY + 80] = v_y_r[:, None]
    tgr = (T_GRID[TT_IDX] - reax).astype(f32)
    xin[:, _C_TG:_C_TG + 40] = np.where(mlz > 0, tgr, f32(-1e30))[None, :]
    xin[:, _C_MA:_C_MA + 40] = (msk * mlz)[None, :]   # col 0: shift==1 lane
    xin[:, _C_ONES:_C_ONES + P] = 1.0
    sc = _C_SC
    xin[:, sc + 0], xin[:, sc + 1] = sm, -sm
    xin[:, sc + 2], xin[:, sc + 3] = inv_am, -inv_am
    xin[:, sc + 4], xin[:, sc + 5] = f32(2.0) * am, -(f32(1.0) / sm)
    xin[:, sc + 6] = np.sqrt(f32(1.0) / (f32(2.0) * am * sm))
    xin[:, sc + 7] = sm * sm
    xin[:, sc + 8] = -f32(f32(3.14) / (f32(1.732) * sig))
    xin[:, sc + 9] = lo * teams + ld * (f32(1.0) - teams)
    return xin


def kernel(frame, s_max, a_max, tti_sigma, tti_lambda_off, tti_lambda_def):
    from concourse import bass_utils

    frame = np.asarray(frame, dtype=f32)
    B = frame.shape[0]
    nc = _get_nc()
    out = np.zeros((B, P), f32)
    for b in range(B):
        xin = _host_prep(frame[b:b + 1], s_max, a_max, tti_sigma,
                         tti_lambda_off, tti_lambda_def)
        in_maps = [{"xin": xin} for _ in range(N_CORES)]
        res = bass_utils.run_bass_kernel_spmd(nc, in_maps,
                                              core_ids=list(range(N_CORES)))
        out[b] = res.results[0]["res"][:, 0]
    return out



# revision 14
# speedup vs baseline: 1.5680x; 1.5680x over previous
"""Trainium2 Bass kernel for nn_CompProbModel_42691974922925.

Reference semantics: for each batch frame, the model builds a completion-
probability field over F=6600 field cells x NT=40 pass durations x P=10
players, then gathers a single row ``out = ind_pass[b_idx, tof, :]`` where
``b_idx`` (ball target cell) and ``tof`` (time-of-flight index) are scalars
derived from the frame. Exact dead-code elimination: the gathered row only
depends on the 40 trajectory cells ``path[b_idx, tof, s]`` (s = traj step),
so the live computation is a [40 steps x 10 players] problem:

    p[s,p]    = sigmoid(c * (T[tt_idx[s]] - t_tot(cell_s, player_p))) * lam_z
    q[s]      = max(1, sum_p p[s,p]);  pn = p / q
    all_t[s]  = sum_p pn[s,p]
    rem       = cumprod_s(1 - all_t);  shift = roll(rem, 1), shift[0] = 1
    out[p]    = sum_{s<=tof} shift[s] * pn[s,p] * lam_all[p]

Host side (numpy, f32-exact vs the jax reference): index math (tof, b_idx,
trajectory cell indices via round-half-even), gathering FIELD_LOCS rows and
packing operand blocks. Device side (Bass/Tile, per core): all the real
arithmetic - kinematics distances, both square roots, the sigmoid, the
normalization, the exact cumprod survival scan, and the final contraction.

Device-side structure (all engines see a [P=10 partitions, free] layout):

- TWO activation tables, one reload: the program starts on the
  ``sqrt_and_others`` table (both square roots run as single table ops -
  ``2*am*dmag = Sqrt(4*am^2 * d2)`` folds the 2*am scale into the Sqrt's
  input scale), then one explicit LoadActFuncSet switches to
  ``sigmoid_and_others`` for the exact-table Sigmoid and the Relu.  The
  1283ns reload overlaps the (off-critical-path) speed-branch arithmetic.
- The time-to-target math is algebraically compressed (as in the reference
  torch source): speed-limited branch  t_tot - reax = dmag/sm +
  (sm-s0)^2/(2*am*sm); the branch condition d_lt > d_mag is exactly
  w1 < sm^2 with w1 = clip(s0)^2 + 2*am*dmag, and clip(s0)^2 =
  min(s0_unclipped^2, sm^2) = min(num^2/d2, sm^2) comes off a DVE
  Reciprocal - so w1 (the second Sqrt's operand) needs only ONE activation
  hop (the first Sqrt) before it, not a full rsqrt->s0->clip->square chain.
- The catchability window lam_z folds into the host-packed time row
  (masked lanes get tgr = -1e4, driving the sigmoid to exactly 0), and the
  tril(tof) selector folds into the survival scan: tensor_tensor_scan
  computes state = state * v[s] * msk[s+1], which telescopes to
  rem[s]*msk[s+1] because the tril row is a monotone step mask.
- The player sum is one PE matmul against an all-ones [P,P] block (row sum
  replicated across partitions); normalization uses min(1, 1/S) == 1/max(1,S)
  so the PSUM reciprocal fuses with the min into one scalar_tensor_tensor.
- The output skips the 2.2us HWDGE DMA path entirely: a KV-writeback DMA is
  descriptor-prepared on the idle Pool engine during the compute phase
  (SWDGE PREPARE_ONLY), and when the result lands a trigger_dma fires the
  pre-generated descriptor - the tail is trigger + transfer + DMA-semaphore
  propagation (~1us) instead of HWDGE generation + DGE delay (~2.3us).

Sharding across the 8 NeuronCores: the live problem after the trajectory
reduction is tiny and sequential (cumprod over s), so inputs are replicated
and every core computes the full result redundantly; core 0's output is
returned. (The [F,40,40,P] field sweep the sharding hint describes is dead
code for the final gather, so there is nothing left worth splitting.)
"""

import numpy as np

f32 = np.float32
NX, NY, NT, P = 120, 55, 40, 10
F = NX * NY
G = 10.72468

# T_GRID = jnp.linspace(0.1, 4.0, 40, dtype=float32) - exact bits as produced
# by jax (identical on the CPU and neuron backends; np.linspace differs by
# 1 ulp at 6 entries, so the bit pattern is pinned here).
_TGRID_BITS = [
    0x3DCCCCCD, 0x3E4CCCCD, 0x3E99999A, 0x3ECCCCCD, 0x3F000000, 0x3F19999A,
    0x3F333334, 0x3F4CCCCD, 0x3F666667, 0x3F800000, 0x3F8CCCCD, 0x3F99999A,
    0x3FA66667, 0x3FB33334, 0x3FC00000, 0x3FCCCCCD, 0x3FD9999A, 0x3FE66666,
    0x3FF33333, 0x40000000, 0x40066667, 0x400CCCCD, 0x40133334, 0x4019999A,
    0x40200000, 0x40266667, 0x402CCCCD, 0x40333334, 0x4039999A, 0x40400000,
    0x40466667, 0x404CCCCD, 0x40533333, 0x4059999A, 0x40600000, 0x40666666,
    0x406CCCCD, 0x40733333, 0x4079999A, 0x40800000,
]
T_GRID = np.array(_TGRID_BITS, dtype=np.uint32).view(f32)

_x = np.linspace(0.5, 119.5, 120)
_y = np.linspace(-0.5, 53.5, 55)
_y[0] = -0.2
_yy, _xx = np.meshgrid(_y, _x, indexing="ij")
FIELD_LOCS = np.stack([_xx, _yy], -1).reshape(-1, 2).astype(f32)  # [6600,2]

# tt_idx[s] = round(10*T[s]-1): traj-step -> p_int time index (== arange(40)
# for these bits, but computed generically).
TT_IDX = np.round(f32(10.0) * T_GRID - f32(1.0)).astype(np.int32)

N_CORES = 8

# single packed device input [P, XCOLS] (f32); column map:
_C_CXY = 0      # 0:80    field-cell coords along traj (cx | cy), bcast over p
_C_RVXY = 80    # 80:160  player reaction velocities (rvx*40 | rvy*40)
_C_RLXY = 160   # 160:240 player reaction locations (rlx*40 | rly*40)
_C_TG = 240     # 240:280 tgr[s] = T[tt_idx[s]] - reax, or -1e4 on lam_z==0
_C_M2 = 280     # 280:319 msk2[t] = [t+1 <= tof] : the scan's step-mask row
_C_W2 = 320     # 320:360 w2sh tile: col 0 = 1.0 (shift[0] lane), cols 1:40
                #         get the survival scan's output in place
_C_ONES = 360   # 360:370 ones (the [P,P] block for the PE broadcast-sum)
_C_SC = 370     # 370:381 scalars, see _host_prep
XCOLS = 384

_CACHE = {}


def _compile_with_two_tables(nc):
    """Steer the act-table pass to exactly two sets: index 3
    (sqrt_and_others) serving Sqrt, and index 2 (sigmoid_and_others)
    serving Sigmoid + Relu. The pass then emits the initial load of set 3
    (overlapping the input DMA) and ONE mid-program reload to set 2,
    placed directly before the first Sigmoid in the activation stream.
    Ids stay aligned with act_info.json; only the membership is narrowed.
    Falls back to stock tables if the arch's table list doesn't match."""
    import concourse.bacc as bacc
    import concourse.mybir as mybir

    act = mybir.ActivationFunctionType
    orig = bacc.get_activation_tables

    def patched(arch):
        tabs = list(orig(arch).items())
        if (len(tabs) > 3 and act.Sqrt in tabs[3][1]
                and {act.Sigmoid, act.Relu} <= tabs[2][1]):
            out = {}
            for i, (name, funcs) in enumerate(tabs):
                if i == 3:
                    out[name] = {act.Sqrt}
                elif i == 2:
                    out[name] = {act.Sigmoid, act.Relu}
                else:
                    out[name] = set()
            return out
        return dict(tabs)

    bacc.get_activation_tables = patched
    try:
        nc.compile()
    finally:
        bacc.get_activation_tables = orig


def _strip_sync_overheads(nc):
    """Post-compile IR surgery on the fixed-overhead regions the tile
    template inserts around the (tiny) body. All of it changes the program
    that actually runs (the NEFF is compiled from this module), not just
    the cost model's view of it.

    1. The entry all-engine barrier only orders the const-pool memsets
       against their first reader (already ordered by the input-DMA data
       dependency), so it is dropped; SP's barrier-piece instructions are
       deleted outright so the input DMA issues at t~0.
    2. The exit drain + double all-engine barrier collapses to just the
       semaphore range-clear (needed so the NEXT invocation of the same
       NEFF starts with zeroed semaphores). It moves to SP directly after
       the body's final wait on the writeback-DMA completion semaphore:
       SP program order then guarantees the clear runs after every
       semaphore's final update has been consumed. Keeping Pool's exit
       stream free of parked waits also lets the trigger's deferred DMA
       timeline acquire Pool.SEQ in the cost model (and mirrors the real
       ring hand-off, which needs no Pool sequencer involvement).
    """
    import bass_rust as _br
    import concourse.mybir as mybir

    fn = nc.m.functions[0]
    blocks = fn.blocks
    assert len(blocks) == 3, [b.name for b in blocks]
    b0, b1, b2 = blocks

    def neuter(inst):
        # Drains carry no sync; EventSemaphores need at least one field for
        # walrus codegen, so they get an always-true >=0 wait.
        if inst.opcode == "EventSemaphore":
            si = inst.sync_info
            sem_id = (si.on_wait + si.on_update)[0].id if si else 152
            inst.sync_info = _br.SyncInfo(on_wait=[_br.SyncWait(
                sync_type="semaphore", id=sem_id, ant_name="noop",
                wait_mode="sem-ge-imm", wait_value=0, wait_reg=None,
            )], on_update=[])
        else:
            inst.sync_info = None

    # Entry: neuter everything; delete SP's pieces so the DMA leads.
    sp_engine = None
    for inst in b1.instructions:
        if inst.opcode == "DMACopy":
            sp_engine = inst.engine
            break
    assert sp_engine is not None
    keep0 = []
    for inst in b0.instructions:
        if inst.opcode in ("Drain", "EventSemaphore"):
            if inst.engine == sp_engine:
                continue  # delete: SP must reach the input DMACopy at t~0
            neuter(inst)
        keep0.append(inst)
    b0.instructions = keep0

    # The final accum op ends the chained DVE lane protocol (wait >= k,
    # inc +1); lane value k+1 therefore means "res128 is written".
    accum = next(i for i in reversed(b1.instructions)
                 if i.opcode == "TensorScalarPtr"
                 and "res128" in str(i.outs))
    acc_upd = next(u for u in accum.sync_info.on_update
                   if u.update_mode == "sem-inc")
    acc_wait = next(w for w in accum.sync_info.on_wait
                    if w.id == acc_upd.id)
    res_done = _br.SyncWait(
        sync_type="semaphore", id=acc_upd.id, ant_name=acc_upd.ant_name,
        wait_mode="sem-ge-imm",
        wait_value=acc_wait.wait_value + acc_upd.update_value, wait_reg=None)

    # Tile attributes the writeback's deferred SBUF read to the PREP and
    # puts write-after-read guards (waits on its DMASW lane sem) in front
    # of the res128 accum op and the trigger. That is circular here - the
    # accum op IS the producer the DMA waits for - and the DMASW sem is
    # never bumped anyway (a prepare_only descriptor fires only the baked
    # sem=). The DVE-side guard is vacuous (nothing touches res128 after
    # the accum): neuter it. The Pool-side guard sits directly before the
    # trigger, which has only one HW wait slot (already spent on the
    # prep's desc-gen tick) - so repurpose the guard to carry the missing
    # "res128 written" gate: the deferred source read must not fire until
    # the accum lands, and Tile's auto-dep misses it (the accum was
    # emitted after the prep).
    for blk in blocks:
        for inst in blk.instructions:
            si = inst.sync_info
            if si is None or not si.on_wait:
                continue
            if any((w.ant_name or "").startswith("DMASW") and w.wait_value
                   for w in si.on_wait):
                new_waits = [w for w in si.on_wait
                             if not ((w.ant_name or "").startswith("DMASW")
                                     and w.wait_value)]
                if not new_waits and inst.opcode == "EventSemaphore":
                    if str(inst.engine) == "EngineType.Pool":
                        new_waits = [res_done]
                    else:
                        new_waits = [_br.SyncWait(
                            sync_type="semaphore", id=si.on_wait[0].id,
                            ant_name="noop", wait_mode="sem-ge-imm",
                            wait_value=0, wait_reg=None)]
                inst.sync_info = _br.SyncInfo(
                    on_wait=new_waits, on_update=list(si.on_update))

    # The compile pass drops the initial LoadActFuncSet directly before the
    # first Activation - which sits BEHIND the Tile-split EventSemaphore
    # that waits for the input DMA (the Sqrt's scale operand). The load has
    # no dependencies, so hoist it ahead of that wait: it then overlaps the
    # input DMA instead of adding 1283ns to the critical path.
    act_engine = next(i.engine for i in b1.instructions
                      if i.opcode == "Activation")
    act_idx = [k for k, i in enumerate(b1.instructions)
               if i.engine == act_engine
               and i.opcode in ("EventSemaphore", "LoadActFuncSet",
                                "Activation")]
    first_load = next(k for k in act_idx
                      if b1.instructions[k].opcode == "LoadActFuncSet")
    first_wait = next(k for k in act_idx
                      if b1.instructions[k].opcode == "EventSemaphore"
                      and b1.instructions[k].sync_info is not None
                      and any((w.ant_name or "").startswith("DMA")
                              for w in b1.instructions[k].sync_info.on_wait))
    if first_wait < first_load:
        load_inst = b1.instructions.pop(first_load)
        b1.instructions.insert(first_wait, load_inst)

    # Tile's cross-engine wait dedup gates the DVE's PSUM reciprocal
    # (reads PSUM written by the matmul) on the activation engine's relu
    # tick instead of the PE semaphore, serializing the two parallel arms
    # of the tail. Rewrite it to wait on the matmul directly.
    mm = next(i for i in b1.instructions if i.opcode == "Matmult")
    mm_upd = mm.sync_info.on_update[0]
    recip = next(i for i in b1.instructions if i.opcode == "Reciprocal"
                 and "ps_sum" in str(i))
    recip.sync_info = _br.SyncInfo(on_wait=[_br.SyncWait(
        sync_type="semaphore", id=mm_upd.id, ant_name=mm_upd.ant_name,
        wait_mode="sem-ge-imm", wait_value=1, wait_reg=None,
    )], on_update=list(recip.sync_info.on_update))

    # Exit: keep only the semaphore range-clear, moved to SP with no sync
    # (SP program order after the body's dma-sem wait is enough); delete
    # the drains and barrier pieces outright so no engine parks and the SP
    # tail costs no sequencer time after the DMA semaphore lands.
    keep2 = []
    for inst in b2.instructions:
        if getattr(inst, "is_reset_sema", False) \
                or "RANGE_CLEAR" in inst.opcode.upper():
            inst.engine = sp_engine
            inst.sync_info = None
            keep2.append(inst)
        elif inst.opcode in ("Drain", "EventSemaphore"):
            continue
        else:
            keep2.append(inst)
    b2.instructions = keep2


def _build_program():
    """Build + compile the 8-core Bass program once per process."""
    import concourse.bacc as bacc
    import concourse.bass as bass
    import concourse.mybir as mybir
    import concourse.tile as tile

    dt = mybir.dt.float32
    op = mybir.AluOpType
    act = mybir.ActivationFunctionType

    nc = bacc.Bacc("TRN2", target_bir_lowering=False, debug=False,
                   num_devices=N_CORES)
    x_dram = nc.dram_tensor("xin", [P, XCOLS], dt, kind="ExternalInput")
    out_dram = nc.dram_tensor("res", [1, 128, 1, 1], dt,
                              kind="ExternalOutput")

    with tile.TileContext(nc) as tc:
        with (
            tc.tile_pool(name="sb", bufs=1) as pool,
            tc.tile_pool(name="ps", bufs=1, space=bass.MemorySpace.PSUM) as psp,
        ):
            x = pool.tile([P, XCOLS], dt, name="x", tag="x")
            nc.sync.dma_start(x[:], x_dram.ap())

            cxy = x[:, _C_CXY:_C_CXY + 80]
            rvxy = x[:, _C_RVXY:_C_RVXY + 80]
            rlxy = x[:, _C_RLXY:_C_RLXY + 80]
            tgr = x[:, _C_TG:_C_TG + 40]
            msk2 = x[:, _C_M2:_C_M2 + 39]
            w2sh = x[:, _C_W2:_C_W2 + 40]
            ones_pp = x[:, _C_ONES:_C_ONES + P]
            sco = _C_SC
            sm, negsm = x[:, sco:sco + 1], x[:, sco + 1:sco + 2]
            smsq = x[:, sco + 2:sco + 3]
            fouramsq = x[:, sco + 3:sco + 4]
            inv_2am = x[:, sco + 4:sco + 5]
            inv_am, neg_inv_am = x[:, sco + 5:sco + 6], x[:, sco + 6:sco + 7]
            neg_inv_sm = x[:, sco + 7:sco + 8]
            sqrt_c1 = x[:, sco + 8:sco + 9]
            sigc, lam = x[:, sco + 9:sco + 10], x[:, sco + 10:sco + 11]

            def wt(name, p=P, n=NT, d=dt):
                return pool.tile([p, n], d, name=name, tag=name)

            # Output staging + the SWDGE descriptor prep: both off the
            # critical path, issued first so the Pool engine's ~1.1us
            # descriptor generation overlaps the input DMA + compute.
            res128 = pool.tile([128, 1], dt, name="res128", tag="res128")
            nc.vector.memset(res128[:], 0.0)
            idxs = pool.tile([128, 1], mybir.dt.int32, name="idxs", tag="idxs")
            nc.vector.memset(idxs[:], 0)
            dma_sem = nc.alloc_semaphore("kv_dma")
            nc.gpsimd.kv_writeback(
                out_dram.ap(), res128[:].unsqueeze(1).unsqueeze(1), idxs[:],
                prepare_only=True, sem=dma_sem)

            # kinematics: dxy = cells - rloc -> d2; the velocity dot product
            # and the s0^2 = num^2/d2 reciprocal route run off-path.
            dxy = wt("dxy", n=80)
            nc.vector.tensor_tensor(dxy[:], cxy, rlxy, op.subtract)
            sq = wt("sq", n=80)
            nc.vector.tensor_tensor(sq[:], dxy[:], dxy[:], op.mult)
            d2 = wt("d2")
            nc.vector.tensor_tensor(d2[:], sq[:, 0:40], sq[:, 40:80], op.add)
            # dm2 = 2*am*dmag = Sqrt(4*am^2 * d2): first table-3 hop.
            dm2 = wt("dm2")
            nc.scalar.activation(dm2[:], d2[:], act.Sqrt, scale=fouramsq)
            # off-path: num, num^2, 1/d2, s0^2 (= num^2/d2)
            nm = wt("nm", n=80)
            nc.vector.tensor_tensor(nm[:], dxy[:], rvxy, op.mult)
            num = wt("num")
            nc.vector.tensor_tensor(num[:], nm[:, 0:40], nm[:, 40:80], op.add)
            numsq = wt("numsq")
            nc.vector.tensor_tensor(numsq[:], num[:], num[:], op.mult)
            r2 = wt("r2")
            nc.vector.reciprocal(r2[:], d2[:])
            t_ = wt("t_")
            nc.vector.tensor_tensor(t_[:], numsq[:], r2[:], op.mult)
            nr = wt("nr")
            nc.vector.tensor_tensor(nr[:], num[:], r2[:], op.mult)

            # w1 = clip(s0)^2 + 2*am*dmag = (s0u^2 min sm^2) + dm2 : the
            # second Sqrt's operand, one DVE hop after dm2.
            w1 = wt("w1")
            nc.vector.scalar_tensor_tensor(w1[:], t_[:], smsq, dm2[:],
                                           op.min, op.add)
            rt = wt("rt")
            nc.scalar.activation(rt[:], w1[:], act.Sqrt)

            # speed-branch chain, all off the rt/table-reload critical path:
            dmag = wt("dmag")
            nc.vector.tensor_scalar(dmag[:], dm2[:], inv_2am, None, op.mult)
            q1 = wt("q1")
            nc.vector.scalar_tensor_tensor(q1[:], dmag[:], neg_inv_sm, tgr,
                                           op.mult, op.add)
            s0 = wt("s0")
            nc.vector.scalar_tensor_tensor(s0[:], nr[:], inv_2am, dm2[:],
                                           op.mult, op.mult)
            nc.vector.tensor_scalar(s0[:], s0[:], sm, negsm, op.min, op.max)
            gm = wt("gm", d=mybir.dt.uint8)
            nc.vector.tensor_scalar(gm[:], w1[:], smsq, None, op.is_lt)
            z0 = wt("z0")
            nc.vector.tensor_scalar(z0[:], s0[:], sm, sqrt_c1,
                                    op.subtract, op.mult)
            h = wt("h")
            nc.vector.scalar_tensor_tensor(h[:], s0[:], inv_am, tgr,
                                           op.mult, op.add)
            z1 = wt("z1")
            nc.vector.tensor_tensor(z1[:], z0[:], z0[:], op.mult)
            pm = wt("pm")
            nc.vector.tensor_tensor(pm[:], q1[:], z1[:], op.subtract)
            # accel branch folds onto the critical path after rt:
            pma = wt("pma")
            nc.vector.scalar_tensor_tensor(pma[:], rt[:], neg_inv_am, h[:],
                                           op.mult, op.add)
            nc.vector.copy_predicated(pm[:], gm[:], pma[:])
            # table reload (inserted by the compile pass right here in the
            # activation stream) then the exact-table sigmoid.
            pp = wt("pp")
            nc.scalar.activation(pp[:], pm[:], act.Sigmoid, scale=sigc)

            # player sum, replicated to every partition in one matmul
            ps_sum = psp.tile([P, NT], dt, name="ps_sum", tag="ps_sum")
            nc.tensor.matmul(ps_sum[:], ones_pp, pp[:])
            # survival factor: 1 - S/max(1,S) == relu(1-S) exactly
            v = wt("v")
            nc.scalar.activation(v[:], ps_sum[:], act.Relu,
                                 bias=1.0, scale=-1.0)
            # normalization: 1/max(1,S) == min(1, 1/S); the min fuses into
            # the pp multiply, so PSUM is read just twice (recip + relu).
            # Emitted BEFORE the scan: the DVE engine runs in issue order,
            # and these only need the matmul while the scan also waits for
            # the relu - this order keeps the engine busy in that window.
            recipS = wt("recipS")
            nc.vector.reciprocal(recipS[:], ps_sum[:])
            c2 = wt("c2")
            nc.vector.scalar_tensor_tensor(c2[:], recipS[:], 1.0, pp[:],
                                           op.min, op.mult)
            # fused survival scan + tril mask: state = state*v[s]*msk[s+1]
            # telescopes to rem[s]*msk[s+1] (monotone step mask), written
            # shifted into w2sh cols 1:40 (col 0 is the host-packed 1.0).
            nc.vector.tensor_tensor_scan(w2sh[:, 1:40], v[:, 0:39], msk2,
                                         1.0, op.mult, op.mult)
            # final contraction: res[p] = sum_s c2[p,s]*lam[p]*w2sh[p,s]
            nc.vector.scalar_tensor_tensor(wt("ind")[:], c2[:], lam, w2sh,
                                           op.mult, op.mult,
                                           accum_out=res128[0:P, :])
            # fire the pre-generated writeback descriptor; the surgery below
            # adds a wait on the accum-op's DVE lane tick to the trigger
            # (the prep's deferred source read happens at trigger time, but
            # its auto-dep only covers writers emitted before the prep). SP
            # holds the program open until the DMA-completion sem lands.
            nc.gpsimd.trigger_dma(count=None)
            nc.sync.wait_ge(dma_sem, 16)

    _compile_with_two_tables(nc)
    _strip_sync_overheads(nc)
    return nc


def _get_nc():
    if "nc" not in _CACHE:
        _CACHE["nc"] = _build_program()
    return _CACHE["nc"]


def _host_prep(frame, s_max, a_max, tti_sigma, tti_lambda_off, tti_lambda_def):
    """Index math + operand packing for one batch element (numpy, f32)."""
    fr = np.asarray(frame, dtype=f32)[0]          # [P,13]
    sm = f32(np.asarray(s_max).reshape(-1)[0])
    am = f32(np.asarray(a_max).reshape(-1)[0])
    sig = f32(np.asarray(tti_sigma).reshape(-1)[0])
    lo = f32(np.asarray(tti_lambda_off).reshape(-1)[0])
    ld = f32(np.asarray(tti_lambda_def).reshape(-1)[0])

    reax = f32(sm / am)
    v_x_r = fr[:, 5] * reax + fr[:, 3]
    v_y_r = fr[:, 6] * reax + fr[:, 4]
    x_r = fr[:, 1] + fr[:, 3] * reax + f32(0.5) * fr[:, 5] * f32(reax * reax)
    y_r = fr[:, 2] + fr[:, 4] * reax + f32(0.5) * fr[:, 6] * f32(reax * reax)
    teams = fr[:, 7]
    rlx = x_r.astype(np.int32).astype(f32)        # trunc-toward-zero like jax
    rly = y_r.astype(np.int32).astype(f32)

    # scalar gathers (match jax negative-index wrap + OOB clip semantics)
    tof = int(np.round(fr[0, 12])) - 1
    if tof < 0:
        tof += NT
    tof = min(max(tof, 0), NT - 1)
    b_idx = (int(fr[0, 11]) + 1) * NX + int(fr[0, 10])
    if b_idx < 0:
        b_idx += F
    b_idx = min(max(b_idx, 0), F - 1)

    # ball trajectory for the (b_idx, tof) row; round-half-even like jnp.round
    ball = fr[0, 8:10]
    vx = f32((FIELD_LOCS[b_idx, 0] - ball[0]) / T_GRID[tof])
    vy = f32((FIELD_LOCS[b_idx, 1] - ball[1]) / T_GRID[tof])
    traj_x = np.round(
        np.minimum(np.maximum(ball[0] + vx * T_GRID, f32(0)), f32(NX - 1))
    ).astype(np.int32)
    traj_y = np.round(
        np.minimum(np.maximum(ball[1] + vy * T_GRID, f32(0)), f32(NY - 1))
    ).astype(np.int32)
    path = traj_y * NX + traj_x                    # [40] in-range by clip
    cells = FIELD_LOCS[path]                       # [40,2]

    # catchability window lam_z[tof, s]
    vz0_t = f32(T_GRID[tof] * f32(G) / f32(2.0))
    z_row = f32(2.0) + vz0_t * T_GRID - f32(0.5) * f32(G) * (T_GRID * T_GRID)
    mlz = ((z_row < f32(3.0)) & (z_row > f32(0.0))).astype(f32)

    steps = np.arange(NT)
    msk2 = (steps + 1 <= tof).astype(f32)          # scan's step mask, t=0..38

    xin = np.zeros((P, XCOLS), f32)
    xin[:, _C_CXY:_C_CXY + 40] = cells[:, 0][None, :]
    xin[:, _C_CXY + 40:_C_CXY + 80] = cells[:, 1][None, :]
    xin[:, _C_RLXY:_C_RLXY + 40] = rlx[:, None]
    xin[:, _C_RLXY + 40:_C_RLXY + 80] = rly[:, None]
    xin[:, _C_RVXY:_C_RVXY + 40] = v_x_r[:, None]
    xin[:, _C_RVXY + 40:_C_RVXY + 80] = v_y_r[:, None]
    tgr = (T_GRID[TT_IDX] - reax).astype(f32)
    xin[:, _C_TG:_C_TG + 40] = np.where(mlz > 0, tgr, f32(-1e4))[None, :]
    xin[:, _C_M2:_C_M2 + 39] = msk2[None, :39]
    xin[:, _C_W2] = 1.0                            # shift[0] == 1 lane
    xin[:, _C_ONES:_C_ONES + P] = 1.0
    sc = _C_SC
    xin[:, sc + 0], xin[:, sc + 1] = sm, -sm
    xin[:, sc + 2] = sm * sm
    xin[:, sc + 3] = f32(4.0) * am * am
    xin[:, sc + 4] = f32(1.0) / (f32(2.0) * am)
    inv_am = f32(f32(1.0) / am)
    xin[:, sc + 5], xin[:, sc + 6] = inv_am, -inv_am
    xin[:, sc + 7] = -(f32(1.0) / sm)
    xin[:, sc + 8] = np.sqrt(f32(1.0) / (f32(2.0) * am * sm))
    xin[:, sc + 9] = f32(f32(3.14) / (f32(1.732) * sig))
    xin[:, sc + 10] = lo * teams + ld * (f32(1.0) - teams)
    return xin


def kernel(frame, s_max, a_max, tti_sigma, tti_lambda_off, tti_lambda_def):
    from concourse import bass_utils

    frame = np.asarray(frame, dtype=f32)
    B = frame.shape[0]
    nc = _get_nc()
    out = np.zeros((B, P), f32)
    for b in range(B):
        xin = _host_prep(frame[b:b + 1], s_max, a_max, tti_sigma,
                         tti_lambda_off, tti_lambda_def)
        in_maps = [{"xin": xin} for _ in range(N_CORES)]
        res = bass_utils.run_bass_kernel_spmd(nc, in_maps,
                                              core_ids=list(range(N_CORES)))
        out[b] = res.results[0]["res"].reshape(128)[:P]
    return out
